# revision 38
# baseline (speedup 1.0000x reference)
"""DBRX MoE router kernel for 8x Trainium2 NeuronCores (Bass/Tile).

Computes, for x [4, 4096, 6144] f32 and W [6144, 16] f32:
    logits      = x @ W                      (per-token skinny GEMM, K=6144, E=16)
    weights     = softmax(logits, axis=-1)
    top_w, top_e = top_k(weights, 4)
    top_w       = top_w / sum(top_w)         (L1 renorm)
returns (weights f32 [4,4096,16], top_w f32 [4,4096,4], top_e int32 [4,4096,4])

Sharding: data-parallel over tokens. The 16384 tokens are split into 8
contiguous shards of 2048; W is replicated. All ops are token-local.

Layout/precision strategy:
- The host pre-transposes each core's shard to x^T [4 groups, 6144 d, tokens]
  so the device streams fully-contiguous 2 MB DMAs with the contraction dim
  on partitions, split across both HWDGE rings (sync + scalar).
- fp32 is decomposed hi/lo into two fp16 halves packed in the same 4 bytes
  (h1 = fp16(x), h2 = fp16((x - h1) * 4096)), so DMA volume is unchanged but
  TensorE runs fp16 at 1 cycle/row instead of fp32's 4 (fp32 = 2 half-rate
  passes on trn2). logits = h1@W1 + (h2@W1 + h1@W2) / 4096 with W split the
  same way recovers full fp32-class precision (the dropped lo*lo term is
  ~2^-24 relative).
- TensorE accumulates logits^T [16, 512] per token group (W chunk stationary,
  512 tokens moving; the correction stream goes to a second PSUM bank and is
  scaled in at the epilogue), then transposes 128-token tiles to [128, 16]
  via identity matmul for the softmax/top-k epilogue on ScalarE (Exp with
  fused accumulated sum) and VectorE (reciprocal, scale, hw max8/max8-index).
"""

import os
import numpy as np
from concurrent.futures import ThreadPoolExecutor

import concourse.bass as bass
import concourse.bacc as bacc
import concourse.mybir as mybir
import concourse.tile as tile
from concourse.bass import ds, ts
from concourse.bass_utils import run_bass_kernel_spmd
from concourse.masks import make_identity

# Problem constants (hardcoded per harness contract)
B, S, D, E = 4, 4096, 6144, 16
TOPK = 4
N_CORES = 8
TOKENS = B * S                  # 16384
TPC = TOKENS // N_CORES         # 2048 tokens per core
NGROUP = 4                      # token groups per core
GTOK = TPC // NGROUP            # 512 tokens per group
NTILE = GTOK // 128             # 4 token tiles (128 tokens) per group
KCHUNK = D // 128               # 48 contraction chunks
CH_PER_DMA = 8                  # k-chunks per input DMA (8 * 128 * 512 * 4B = 2 MB)
NDMA = KCHUNK // CH_PER_DMA     # 6 DMAs per group
SC = 4096.0                     # lo-half scale (2**12)

_CACHE = {}


def _build_program():
    nc = bacc.Bacc("TRN2", target_bir_lowering=False)

    f32 = mybir.dt.float32
    f16 = mybir.dt.float16
    u32 = mybir.dt.uint32

    xt = nc.dram_tensor("xt", [NGROUP, D, 2 * GTOK], f16, kind="ExternalInput")
    w1 = nc.dram_tensor("w1", [128, KCHUNK * E], f16, kind="ExternalInput")
    w2 = nc.dram_tensor("w2", [128, KCHUNK * E], f16, kind="ExternalInput")
    # outputs in partition-major layout [p, g, t, e]; the host un-permutes.
    # This gives the store DMAs long contiguous per-partition lines instead
    # of 64 B token rows.
    out_w = nc.dram_tensor("out_w", [128, NGROUP * NTILE * E], f32, kind="ExternalOutput")
    out_tw = nc.dram_tensor("out_tw", [128, NGROUP * NTILE * TOPK], f32, kind="ExternalOutput")
    out_te = nc.dram_tensor("out_te", [128, NGROUP * NTILE * TOPK], u32, kind="ExternalOutput")

    with tile.TileContext(nc) as tc:
        with (
            tc.tile_pool(name="wpool", bufs=1) as wpool,
            tc.tile_pool(name="xpool", bufs=7) as xpool,
            tc.tile_pool(name="psmain", bufs=2, space="PSUM") as psmain_pool,
            tc.tile_pool(name="pscorr", bufs=2, space="PSUM") as pscorr_pool,
            tc.tile_pool(name="pstr", bufs=2, space="PSUM") as pstr_pool,
            tc.tile_pool(name="epool", bufs=2) as epool,
            tc.tile_pool(name="opool", bufs=2) as opool,
        ):
            w1_sb = wpool.tile([128, KCHUNK * E], f16)
            nc.gpsimd.dma_start(out=w1_sb, in_=w1[:, :])
            w2_sb = wpool.tile([128, KCHUNK * E], f16)
            nc.gpsimd.dma_start(out=w2_sb, in_=w2[:, :])
            ident = wpool.tile([E, E], f32)
            make_identity(nc, ident)
            w_all = wpool.tile([128, NGROUP, NTILE, E], f32)
            tw_all = wpool.tile([128, NGROUP, NTILE, TOPK], f32)
            te_all = wpool.tile([128, NGROUP, NTILE, TOPK], u32)

            for g in range(NGROUP):
                # ---- stream packed x^T for this token group across both
                # HWDGE rings; the last group uses finer pieces so the final
                # matmul burst after the last byte lands is short
                pieces = [CH_PER_DMA] * NDMA
                xtiles = []
                c0 = 0
                for h, npc in enumerate(pieces):
                    xt_sb = xpool.tile([128, CH_PER_DMA, 2 * GTOK], f16)
                    src = xt[g, ds(c0 * 128, npc * 128), :]
                    src = src.rearrange("(j p) t -> p j t", p=128)
                    eng = nc.sync if (h + g) % 2 == 0 else nc.scalar
                    eng.dma_start(out=xt_sb[:, 0:npc, :], in_=src)
                    xtiles.append((xt_sb, c0, npc))
                    c0 += npc

                # ---- logits^T [16, 512]: 3 fp16 streams, W chunk stationary
                main_ps = psmain_pool.tile([E, GTOK], f32)
                corr_ps = pscorr_pool.tile([E, GTOK], f32)
                for xt_sb, c0, npc in xtiles:
                    for j in range(npc):
                        c = c0 + j
                        ar = xt_sb[:, j, :].rearrange("p (t two) -> p two t", two=2)
                        a1, a2 = ar[:, 0, :], ar[:, 1, :]
                        first, last = c == 0, c == KCHUNK - 1
                        nc.tensor.matmul(
                            main_ps, w1_sb[:, ts(c, E)], a1, start=first, stop=last
                        )
                        nc.tensor.matmul(
                            corr_ps, w1_sb[:, ts(c, E)], a2, start=first, stop=False
                        )
                        nc.tensor.matmul(
                            corr_ps, w2_sb[:, ts(c, E)], a1, start=False, stop=last
                        )

                # ---- combine streams + transpose logits to [128 tok, 16]
                corr_sb = epool.tile([E, GTOK], f32)
                nc.scalar.activation(
                    corr_sb, corr_ps, mybir.ActivationFunctionType.Copy, scale=1.0 / SC
                )
                logT_sb = epool.tile([E, GTOK], f32)
                nc.vector.tensor_add(logT_sb, corr_sb, main_ps)
                logits_ps = pstr_pool.tile([128, NTILE * E], f32)
                for t in range(NTILE):
                    nc.tensor.transpose(
                        logits_ps[:, ts(t, E)], logT_sb[:, ts(t, 128)], ident
                    )

                # ---- softmax + top-4 epilogue
                e_sb = epool.tile([128, NTILE, E], f32)
                s_sb = epool.tile([128, NTILE], f32)
                r_sb = epool.tile([128, NTILE], f32)
                m8 = epool.tile([128, NTILE, 8], f32)
                i8 = epool.tile([128, NTILE, 8], u32)
                s4 = epool.tile([128, NTILE], f32)
                r4 = epool.tile([128, NTILE], f32)
                w_out = w_all[:, g, :, :]
                tw_out = tw_all[:, g, :, :]
                te_out = te_all[:, g, :, :]

                for t in range(NTILE):
                    # e = exp(logits) on ScalarE
                    nc.scalar.activation(
                        e_sb[:, t, :],
                        logits_ps[:, ts(t, E)],
                        mybir.ActivationFunctionType.Exp,
                    )
                # batched softmax denominators on VectorE
                nc.vector.tensor_reduce(
                    s_sb, e_sb, axis=mybir.AxisListType.X, op=mybir.AluOpType.add
                )
                nc.vector.reciprocal(r_sb, s_sb)
                for t in range(NTILE):
                    # full softmax output
                    nc.vector.tensor_scalar_mul(
                        w_out[:, t, :], e_sb[:, t, :], r_sb[:, ds(t, 1)]
                    )
                    # hardware top-8 (we use the top 4)
                    nc.vector.max(out=m8[:, t, :], in_=w_out[:, t, :])
                    nc.vector.max_index(
                        out=i8[:, t, :], in_max=m8[:, t, :], in_values=w_out[:, t, :]
                    )
                # L1 renorm of the top-4
                nc.vector.tensor_reduce(
                    s4, m8[:, :, 0:TOPK], axis=mybir.AxisListType.X,
                    op=mybir.AluOpType.add,
                )
                nc.vector.reciprocal(r4, s4)
                for t in range(NTILE):
                    nc.vector.tensor_scalar_mul(
                        tw_out[:, t, :], m8[:, t, 0:TOPK], r4[:, ds(t, 1)]
                    )
                nc.vector.tensor_copy(te_out, i8[:, :, 0:TOPK])

                # ---- store this group's outputs: contiguous per-partition
                # lines in the p-major DRAM layout, overlapped with the stream
                nc.sync.dma_start(
                    out=out_w[:, ds(g * NTILE * E, NTILE * E)], in_=w_out
                )
                nc.scalar.dma_start(
                    out=out_tw[:, ds(g * NTILE * TOPK, NTILE * TOPK)], in_=tw_out
                )
                nc.sync.dma_start(
                    out=out_te[:, ds(g * NTILE * TOPK, NTILE * TOPK)], in_=te_out
                )
    nc.finalize()
    return nc


def _prep_core_input(x_flat, core):
    shard = x_flat[core * TPC:(core + 1) * TPC]          # [2048, 6144] view
    shard = shard.reshape(NGROUP, GTOK, D)               # [4, 512, 6144] view
    xt = np.ascontiguousarray(shard.transpose(0, 2, 1))  # [4, 6144, 512] f32
    h1 = xt.astype(np.float16)
    h2 = ((xt - h1.astype(np.float32)) * SC).astype(np.float16)
    packed = np.empty((NGROUP, D, 2 * GTOK), np.float16)
    packed[:, :, 0::2] = h1
    packed[:, :, 1::2] = h2
    return packed


def _run(x, W, trace=False, tmpdir=None):
    x = np.asarray(x, dtype=np.float32)
    W = np.asarray(W, dtype=np.float32)
    if "nc" not in _CACHE:
        _CACHE["nc"] = _build_program()
    nc = _CACHE["nc"]

    x_flat = x.reshape(TOKENS, D)
    # W pre-chunked: [p, c*16+e] = W[128c+p, e], split hi/lo like x
    wprep = np.ascontiguousarray(
        W.reshape(KCHUNK, 128, E).transpose(1, 0, 2)
    ).reshape(128, KCHUNK * E)
    w1 = wprep.astype(np.float16)
    w2 = ((wprep - w1.astype(np.float32)) * SC).astype(np.float16)

    with ThreadPoolExecutor(N_CORES) as ex:
        shards = list(ex.map(lambda c: _prep_core_input(x_flat, c), range(N_CORES)))

    in_maps = [{"xt": shards[c], "w1": w1, "w2": w2} for c in range(N_CORES)]
    res = run_bass_kernel_spmd(
        nc, in_maps, core_ids=list(range(N_CORES)), trace=trace, tmpdir=tmpdir
    )

    def unperm(name, width, dtype):
        # device layout [p, g, t, e] -> tokens (g, t, p) row-major
        parts = []
        for r in res.results:
            a = r[name].reshape(128, NGROUP, NTILE, width)
            parts.append(np.transpose(a, (1, 2, 0, 3)).reshape(TPC, width))
        return np.concatenate(parts).astype(dtype, copy=False)

    weights = unperm("out_w", E, np.float32).reshape(B, S, E)
    top_w = unperm("out_tw", TOPK, np.float32).reshape(B, S, TOPK)
    top_e = unperm("out_te", TOPK, np.int32).reshape(B, S, TOPK)
    return (weights, top_w, top_e), res


def kernel(x, W):
    out, _ = _run(x, W)
    return out


# revision 39
# speedup vs baseline: 1.0716x; 1.0716x over previous
"""DBRX MoE router kernel for 8x Trainium2 NeuronCores (Bass/Tile).

Computes, for x [4, 4096, 6144] f32 and W [6144, 16] f32:
    logits      = x @ W                      (per-token skinny GEMM, K=6144, E=16)
    weights     = softmax(logits, axis=-1)
    top_w, top_e = top_k(weights, 4)
    top_w       = top_w / sum(top_w)         (L1 renorm)
returns (weights f32 [4,4096,16], top_w f32 [4,4096,4], top_e int32 [4,4096,4])

Sharding: data-parallel over tokens. The 16384 tokens are split into 8
contiguous shards of 2048; W is replicated. All ops are token-local.

Layout/precision strategy:
- The host pre-transposes each core's shard to x^T [4 groups, 6144 d, tokens]
  so the device streams fully-contiguous 2 MB DMAs with the contraction dim
  on partitions, split across both HWDGE rings (sync + scalar).
- fp32 is decomposed hi/lo into two fp16 halves packed in the same 4 bytes
  (h1 = fp16(x), h2 = fp16((x - h1) * 4096)), so DMA volume is unchanged but
  TensorE runs fp16 at 1 cycle/row instead of fp32's 4 (fp32 = 2 half-rate
  passes on trn2). logits = h1@W1 + (h2@W1 + h1@W2) / 4096 with W split the
  same way recovers full fp32-class precision (the dropped lo*lo term is
  ~2^-24 relative).
- TensorE accumulates logits^T [16, 512] per token group (W chunk stationary,
  512 tokens moving; the correction stream goes to a second PSUM bank and is
  scaled in at the epilogue), then transposes 128-token tiles to [128, 16]
  via identity matmul for the softmax/top-k epilogue on ScalarE (Exp with
  fused accumulated sum) and VectorE (reciprocal, scale, hw max8/max8-index).
"""

import numpy as np
from concurrent.futures import ThreadPoolExecutor

import concourse.bacc as bacc
import concourse.mybir as mybir
import concourse.tile as tile
from concourse.bass import ds, ts
from concourse.bass_utils import run_bass_kernel_spmd
from concourse.masks import make_identity

# Problem constants (hardcoded per harness contract)
B, S, D, E = 4, 4096, 6144, 16
TOPK = 4
N_CORES = 8
TOKENS = B * S                  # 16384
TPC = TOKENS // N_CORES         # 2048 tokens per core
NGROUP = 4                      # token groups per core
GTOK = TPC // NGROUP            # 512 tokens per group
NTILE = GTOK // 128             # 4 token tiles (128 tokens) per group
KCHUNK = D // 128               # 48 contraction chunks
CH_PER_DMA = 8                  # k-chunks per input DMA (8 * 128 * 512 * 4B = 2 MB)
NDMA = KCHUNK // CH_PER_DMA     # 6 DMAs per group
SC = 4096.0                     # lo-half scale (2**12)

_CACHE = {}


def _build_program():
    nc = bacc.Bacc("TRN2", target_bir_lowering=False)

    f32 = mybir.dt.float32
    f16 = mybir.dt.float16
    u32 = mybir.dt.uint32

    xt = nc.dram_tensor("xt", [NGROUP, D, 2 * GTOK], f16, kind="ExternalInput")
    w1 = nc.dram_tensor("w1", [128, KCHUNK * E], f16, kind="ExternalInput")
    w2 = nc.dram_tensor("w2", [128, KCHUNK * E], f16, kind="ExternalInput")
    # outputs in partition-major layout [p, g, t, e]; the host un-permutes.
    # This gives the store DMAs long contiguous per-partition lines instead
    # of 64 B token rows.
    out_w = nc.dram_tensor("out_w", [128, NGROUP * NTILE * E], f32, kind="ExternalOutput")
    out_tw = nc.dram_tensor("out_tw", [128, NGROUP * NTILE * TOPK], f32, kind="ExternalOutput")
    out_te = nc.dram_tensor("out_te", [128, NGROUP * NTILE * TOPK], u32, kind="ExternalOutput")

    with tile.TileContext(nc) as tc:
        with (
            tc.tile_pool(name="wpool", bufs=1) as wpool,
            tc.tile_pool(name="xpool", bufs=7) as xpool,
            tc.tile_pool(name="psmain", bufs=2, space="PSUM") as psmain_pool,
            tc.tile_pool(name="pscorr", bufs=2, space="PSUM") as pscorr_pool,
            tc.tile_pool(name="pstr", bufs=2, space="PSUM") as pstr_pool,
            tc.tile_pool(name="epool", bufs=2) as epool,
            tc.tile_pool(name="opool", bufs=2) as opool,
        ):
            w1_sb = wpool.tile([128, KCHUNK * E], f16)
            nc.gpsimd.dma_start(out=w1_sb, in_=w1[:, :])
            w2_sb = wpool.tile([128, KCHUNK * E], f16)
            nc.gpsimd.dma_start(out=w2_sb, in_=w2[:, :])
            ident = wpool.tile([E, E], f32)
            make_identity(nc, ident)
            w_all = wpool.tile([128, NGROUP, NTILE, E], f32)
            tw_all = wpool.tile([128, NGROUP, NTILE, TOPK], f32)
            te_all = wpool.tile([128, NGROUP, NTILE, TOPK], u32)

            for g in range(NGROUP):
                # ---- stream packed x^T for this token group across both
                # HWDGE rings; the last group uses finer pieces so the final
                # matmul burst after the last byte lands is short
                if g == NGROUP - 1:
                    pieces = [4] * (KCHUNK // 4)
                else:
                    pieces = [CH_PER_DMA] * NDMA
                xtiles = []
                c0 = 0
                for h, npc in enumerate(pieces):
                    xt_sb = xpool.tile([128, CH_PER_DMA, 2 * GTOK], f16)
                    src = xt[g, ds(c0 * 128, npc * 128), :]
                    src = src.rearrange("(j p) t -> p j t", p=128)
                    eng = nc.sync if (h + g) % 2 == 0 else nc.scalar
                    eng.dma_start(out=xt_sb[:, 0:npc, :], in_=src)
                    xtiles.append((xt_sb, c0, npc))
                    c0 += npc

                # ---- logits^T [16, 512]: 3 fp16 streams, W chunk stationary
                main_ps = psmain_pool.tile([E, GTOK], f32)
                corr_ps = pscorr_pool.tile([E, GTOK], f32)
                for xt_sb, c0, npc in xtiles:
                    for j in range(npc):
                        c = c0 + j
                        ar = xt_sb[:, j, :].rearrange("p (t two) -> p two t", two=2)
                        a1, a2 = ar[:, 0, :], ar[:, 1, :]
                        first, last = c == 0, c == KCHUNK - 1
                        nc.tensor.matmul(
                            main_ps, w1_sb[:, ts(c, E)], a1, start=first, stop=last
                        )
                        nc.tensor.matmul(
                            corr_ps, w1_sb[:, ts(c, E)], a2, start=first, stop=False
                        )
                        nc.tensor.matmul(
                            corr_ps, w2_sb[:, ts(c, E)], a1, start=False, stop=last
                        )

                # ---- combine streams + transpose logits to [128 tok, 16]
                corr_sb = epool.tile([E, GTOK], f32)
                nc.scalar.activation(
                    corr_sb, corr_ps, mybir.ActivationFunctionType.Copy, scale=1.0 / SC
                )
                logT_sb = epool.tile([E, GTOK], f32)
                nc.vector.tensor_add(logT_sb, corr_sb, main_ps)
                logits_ps = pstr_pool.tile([128, NTILE * E], f32)
                for t in range(NTILE):
                    nc.tensor.transpose(
                        logits_ps[:, ts(t, E)], logT_sb[:, ts(t, 128)], ident
                    )

                # ---- softmax + top-4 epilogue
                e_sb = epool.tile([128, NTILE, E], f32)
                s_sb = epool.tile([128, NTILE], f32)
                r_sb = epool.tile([128, NTILE], f32)
                m8 = epool.tile([128, NTILE, 8], f32)
                i8 = epool.tile([128, NTILE, 8], u32)
                s4 = epool.tile([128, NTILE], f32)
                r4 = epool.tile([128, NTILE], f32)
                w_out = w_all[:, g, :, :]
                tw_out = tw_all[:, g, :, :]
                te_out = te_all[:, g, :, :]

                for t in range(NTILE):
                    # e = exp(logits) on ScalarE
                    nc.scalar.activation(
                        e_sb[:, t, :],
                        logits_ps[:, ts(t, E)],
                        mybir.ActivationFunctionType.Exp,
                    )
                # batched softmax denominators on VectorE
                nc.vector.tensor_reduce(
                    s_sb, e_sb, axis=mybir.AxisListType.X, op=mybir.AluOpType.add
                )
                nc.vector.reciprocal(r_sb, s_sb)
                for t in range(NTILE):
                    # full softmax output
                    nc.vector.tensor_scalar_mul(
                        w_out[:, t, :], e_sb[:, t, :], r_sb[:, ds(t, 1)]
                    )
                    # hardware top-8 (we use the top 4)
                    nc.vector.max(out=m8[:, t, :], in_=w_out[:, t, :])
                    nc.vector.max_index(
                        out=i8[:, t, :], in_max=m8[:, t, :], in_values=w_out[:, t, :]
                    )
                # L1 renorm of the top-4
                nc.vector.tensor_reduce(
                    s4, m8[:, :, 0:TOPK], axis=mybir.AxisListType.X,
                    op=mybir.AluOpType.add,
                )
                nc.vector.reciprocal(r4, s4)
                for t in range(NTILE):
                    nc.vector.tensor_scalar_mul(
                        tw_out[:, t, :], m8[:, t, 0:TOPK], r4[:, ds(t, 1)]
                    )
                nc.vector.tensor_copy(te_out, i8[:, :, 0:TOPK])

                # ---- store this group's outputs: contiguous per-partition
                # lines in the p-major DRAM layout, overlapped with the stream
                nc.sync.dma_start(
                    out=out_w[:, ds(g * NTILE * E, NTILE * E)], in_=w_out
                )
                nc.scalar.dma_start(
                    out=out_tw[:, ds(g * NTILE * TOPK, NTILE * TOPK)], in_=tw_out
                )
                nc.sync.dma_start(
                    out=out_te[:, ds(g * NTILE * TOPK, NTILE * TOPK)], in_=te_out
                )
    nc.finalize()
    return nc


def _prep_core_input(x_flat, core):
    shard = x_flat[core * TPC:(core + 1) * TPC]          # [2048, 6144] view
    shard = shard.reshape(NGROUP, GTOK, D)               # [4, 512, 6144] view
    xt = np.ascontiguousarray(shard.transpose(0, 2, 1))  # [4, 6144, 512] f32
    h1 = xt.astype(np.float16)
    h2 = ((xt - h1.astype(np.float32)) * SC).astype(np.float16)
    packed = np.empty((NGROUP, D, 2 * GTOK), np.float16)
    packed[:, :, 0::2] = h1
    packed[:, :, 1::2] = h2
    return packed


def _run(x, W, trace=False, tmpdir=None):
    x = np.asarray(x, dtype=np.float32)
    W = np.asarray(W, dtype=np.float32)
    if "nc" not in _CACHE:
        _CACHE["nc"] = _build_program()
    nc = _CACHE["nc"]

    x_flat = x.reshape(TOKENS, D)
    # W pre-chunked: [p, c*16+e] = W[128c+p, e], split hi/lo like x
    wprep = np.ascontiguousarray(
        W.reshape(KCHUNK, 128, E).transpose(1, 0, 2)
    ).reshape(128, KCHUNK * E)
    w1 = wprep.astype(np.float16)
    w2 = ((wprep - w1.astype(np.float32)) * SC).astype(np.float16)

    with ThreadPoolExecutor(N_CORES) as ex:
        shards = list(ex.map(lambda c: _prep_core_input(x_flat, c), range(N_CORES)))

    in_maps = [{"xt": shards[c], "w1": w1, "w2": w2} for c in range(N_CORES)]
    res = run_bass_kernel_spmd(
        nc, in_maps, core_ids=list(range(N_CORES)), trace=trace, tmpdir=tmpdir
    )

    def unperm(name, width, dtype):
        # device layout [p, g, t, e] -> tokens (g, t, p) row-major
        parts = []
        for r in res.results:
            a = r[name].reshape(128, NGROUP, NTILE, width)
            parts.append(np.transpose(a, (1, 2, 0, 3)).reshape(TPC, width))
        return np.concatenate(parts).astype(dtype, copy=False)

    weights = unperm("out_w", E, np.float32).reshape(B, S, E)
    top_w = unperm("out_tw", TOPK, np.float32).reshape(B, S, TOPK)
    top_e = unperm("out_te", TOPK, np.int32).reshape(B, S, TOPK)
    return (weights, top_w, top_e), res


def kernel(x, W):
    out, _ = _run(x, W)
    return out
